# revision 1
# baseline (speedup 1.0000x reference)
"""GraphTransformerEncoder kernel: data-parallel over graphs across 8 NeuronCores.

Sharding (per spec hint): B=256 graphs -> 8 shards of 32 graphs. All scatters
and the dense per-graph attention are graph-local, so each core computes its
32 graphs independently; outputs are concatenated. Executed via jax/PJRT on
the axon-tunneled trn2 cores (pmap over 8 devices), with CPU fallback.
"""
import numpy as np
import jax
import jax.numpy as jnp
from functools import partial

# Problem constants (hardcoded per contract)
B = 256
N = 128
HID = 256
HEADS = 8
E_PER_G = 2048
EDGE_FEAT = 8
N_GATE_TYPES = 32
MAX_NODES = 256
K_RW = 8
M = 8                    # cores
BS = B // M              # graphs per shard = 32
NS = BS * N              # nodes per shard
ES = BS * E_PER_G        # edges per shard


def _shard_forward(x, sl, dl, gl, edge_attr, edge_gate_type,
                   deg_emb, rw_w, rw_b, Wq, bq, Wk, bk, Wv, bv, Wo, bo,
                   gate_emb, ebp_w, ebp_b):
    """One shard: x [NS,HID]; sl,dl in [0,N); gl in [0,BS) per edge."""
    H = HEADS
    Dh = HID // H

    # positional encoding: in-degree scatter + embedding lookup
    nd = gl * N + dl
    degree = jnp.zeros((NS,), jnp.int32).at[nd].add(1)
    degree = jnp.clip(degree, 0, MAX_NODES - 1)
    pe = deg_emb[degree]

    # random-walk return probabilities per graph
    adj = jnp.zeros((BS, N, N), x.dtype).at[gl, sl, dl].set(1.0)
    adj = ((adj + adj.transpose(0, 2, 1)) > 0).astype(x.dtype)
    deg = adj.sum(axis=2)
    deg_inv = jnp.where(deg > 0, 1.0 / deg, 0.0)
    trans = adj * deg_inv[:, None, :]
    power = jnp.broadcast_to(jnp.eye(N, dtype=x.dtype), (BS, N, N))
    diags = []
    for _ in range(K_RW):
        power = power @ trans
        diags.append(jnp.diagonal(power, axis1=1, axis2=2))
    rw_pe = jnp.stack(diags, axis=-1).reshape(NS, K_RW)
    pe = pe + rw_pe @ rw_w + rw_b

    # edge-aware multi-head attention
    h = (x + pe).reshape(BS, N, HID)
    Q = (h @ Wq + bq).reshape(BS, N, H, Dh)
    K = (h @ Wk + bk).reshape(BS, N, H, Dh)
    V = (h @ Wv + bv).reshape(BS, N, H, Dh)
    scores = jnp.einsum('bihd,bjhd->bhij', Q, K) * (Dh ** -0.5)

    tb = gate_emb[edge_gate_type] + edge_attr @ ebp_w + ebp_b
    bias = jnp.zeros((BS, N, N, H), x.dtype).at[gl, sl, dl].add(tb)
    tb_rev = jnp.where((sl != dl)[:, None], tb, 0.0)
    bias = bias.at[gl, dl, sl].add(tb_rev)
    scores = scores + bias.transpose(0, 3, 1, 2)

    w = jax.nn.softmax(scores, axis=-1)
    out = jnp.einsum('bhij,bjhd->bihd', w, V).reshape(NS, HID)
    return out @ Wo + bo


def kernel(x, edge_index, edge_attr, edge_gate_type, batch,
           deg_emb, rw_w, rw_b, Wq, bq, Wk, bk, Wv, bv, Wo, bo,
           gate_emb, ebp_w, ebp_b):
    x = np.asarray(x, np.float32)
    src = np.asarray(edge_index[0], np.int32)
    dst = np.asarray(edge_index[1], np.int32)
    edge_attr = np.asarray(edge_attr, np.float32)
    edge_gate_type = np.asarray(edge_gate_type, np.int32)

    # --- shard: graphs g in [32m, 32m+32) -> core m (index relabel only) ---
    g = src // N                       # graph id per edge (edges graph-sorted)
    sl = (src - g * N).astype(np.int32)
    dl = (dst - g * N).astype(np.int32)
    gl = (g % BS).astype(np.int32)     # graph id within shard

    xs = x.reshape(M, NS, HID)
    sls = sl.reshape(M, ES)
    dls = dl.reshape(M, ES)
    gls = gl.reshape(M, ES)
    eas = edge_attr.reshape(M, ES, EDGE_FEAT)
    egs = edge_gate_type.reshape(M, ES)

    wts = [np.asarray(a, np.float32) for a in
           (deg_emb, rw_w, rw_b, Wq, bq, Wk, bk, Wv, bv, Wo, bo,
            gate_emb, ebp_w, ebp_b)]

    in_axes = (0, 0, 0, 0, 0, 0) + (None,) * 14

    def run(devs):
        f = jax.pmap(_shard_forward, in_axes=in_axes, devices=devs)
        out = f(xs, sls, dls, gls, eas, egs, *wts)
        return np.asarray(out, np.float32).reshape(B * N, HID)

    try:
        devs = jax.devices()
        assert len(devs) >= M
        return run(devs[:M])
    except Exception:
        cpu = jax.devices('cpu')[0]
        with jax.default_device(cpu):
            outs = [np.asarray(jax.jit(_shard_forward)(
                xs[m], sls[m], dls[m], gls[m], eas[m], egs[m], *wts))
                for m in range(M)]
        return np.concatenate(outs, axis=0).astype(np.float32)

